# revision 1
# baseline (speedup 1.0000x reference)
"""Grouped (MoE-style) linear on 8 trn2 NeuronCores.

out[t] = hidden_states[t] @ weight[g(t)], where token t belongs to group g iff
offsets[g-1] <= t < offsets[g] (searchsorted right semantics; tokens at or past
offsets[-1] get zero output).

Strategy: expert-parallel. Core g owns weight[g] and the contiguous token run
of group g. Routing is done host-side (offsets are host data); each core runs
an identical Bass program: a [P_pad, 1024] x [1024, 1024] matmul tiled as
128-token blocks, contraction in 8 chunks of 128, PSUM-accumulated.

x and w are fed in fp16 (1 cycle/row PE rate like fp32r, half the HBM
traffic); PSUM accumulates fp32; output is written back as fp16 and upcast
host-side (measured end-to-end rel err ~4e-4).

Timeline tuning (from NTFF traces; ~390 GB/s aggregate DMA, ~6.5us fixed
engine preamble, first DMA bytes land ~9us, PE runs at 1.2GHz until ~4.7us
of *uninterrupted* execution then 2.4GHz — any stall resets the ramp):
  - the PE ramps on matmuls over a memset scratch tile starting right after
    the preamble (gated only on the gpsimd memset, not on input DMA);
    NWARM is sized so real matmuls start just after their data lands and
    never outrun the W stream (a stall would reset the p-state ramp).
  - only sync (SP) and scalar (Activation) have hardware DGE queues; each
    drains FIFO so ordering == priority.  sync carries xt0 + W (in the
    exact consumption order of the nb-major first block: all low halves,
    then high halves) then the remaining x; scalar carries w chunk 0, xt1,
    then the per-block output stores.
  - last token block: nb-major into two separate 512-wide PSUM tiles (a
    shared [128,1024] tile would serialize the second group's matmuls
    behind the first group's PSUM->SBUF copies), output drained in
    overlapped 256/512-col pieces.

Host packs per-core inputs so DMA lands with wide contiguous runs per
SBUF partition:
  xt[p, tb, k, tok] = X_g[tb*128 + tok, k*128 + p]   (transposed token block)
  w[p, k, n]        = W_g[k*128 + p, n]
"""
import numpy as np

import concourse.bass as bass
import concourse.tile as tile
from concourse import bacc, mybir
from concourse.bass_utils import run_bass_kernel_spmd

GROUPS = 8
TOKENS = 16384
IN_F = 1024
OUT_F = 1024
KCH = IN_F // 128  # contraction chunks
NWARM = 11         # scratch ramp matmuls (covers m0-ready jitter 10.7-14.3us)


def build(ntb: int) -> bass.Bass:
    """One core's program: ntb 128-token blocks through a 1024x1024 expert."""
    f32 = mybir.dt.float32
    f16 = mybir.dt.float16
    nc = bacc.Bacc()
    xt_d = nc.dram_tensor("xt", [128, ntb, KCH, 128], f16, kind="ExternalInput")
    w_d = nc.dram_tensor("w", [128, KCH, OUT_F], f16, kind="ExternalInput")
    out_d = nc.dram_tensor("out", [ntb * 128, OUT_F], f16, kind="ExternalOutput")

    nind = min(4, ntb)   # token blocks in the early DMAs (xt0 + xt1-3)
    nmid = min(8, ntb)   # token blocks covered by the first bulk DMA
    with tile.TileContext(nc) as tc:
        with (
            tc.tile_pool(name="wp", bufs=1) as wp,
            tc.tile_pool(name="xp", bufs=nind) as xp,
            tc.tile_pool(name="op", bufs=4) as op,
            tc.tile_pool(name="sc", bufs=1) as scp,
            tc.tile_pool(name="ps", bufs=3, space="PSUM") as psp,
            tc.tile_pool(name="ph", bufs=1, space="PSUM") as php,
        ):
            wt = wp.tile([128, KCH, OUT_F], f16)
            sc = scp.tile([128, 640], f16)
            psa = php.tile([128, 512], f32, tag="psa")
            psb = php.tile([128, 512], f32, tag="psb")
            nc.vector.memset(sc[:], 0.0)

            # DMA queues are packet-rate-limited (~130-200 packets/us
            # each) and one packet moves one per-partition contiguous run;
            # aggregate bandwidth caps at ~390 GB/s.  So: few dma_starts,
            # multi-chunk runs (4KB+), but W still split in 3 pieces so
            # chunk semaphores fire before the PE consumes each chunk
            # (first two blocks are interleaved: one chunk per 4 matmuls).
            # sync: xt0, W(k1-2), W(k3-4), W(k5-7), xt1-3, x rest.
            # scalar: w chunk 0 (parallel with xt0), then output stores.
            # Queues run relaxed-ordered (concurrent DMAs share a queue
            # round-robin per packet) and a DMA's completion sem lags its
            # data by ~2us, so the first real matmul cannot start before
            # ~12us no matter what; the scratch ramp below is sized to hand
            # off to it seamlessly at full clock.
            xt0 = xp.tile([128, KCH, 128], f16, tag="xt", bufs=1)
            nc.sync.dma_start(out=xt0[:], in_=xt_d[:, 0])
            nc.scalar.dma_start(out=wt[:, 0, :], in_=w_d[:, 0, :])
            nc.sync.dma_start(out=wt[:, 1:3, :], in_=w_d[:, 1:3, :])
            nc.sync.dma_start(out=wt[:, 3:5, :], in_=w_d[:, 3:5, :])
            nc.sync.dma_start(out=wt[:, 5:, :], in_=w_d[:, 5:, :])
            if nind > 1:
                xtr = xp.tile([128, nind - 1, KCH, 128], f16,
                              tag="xtr", bufs=1)
                nc.sync.dma_start(out=xtr[:], in_=xt_d[:, 1:nind])
            if nmid > nind:
                xmid = xp.tile([128, nmid - nind, KCH, 128], f16,
                               tag="xmid", bufs=1)
                nc.sync.dma_start(out=xmid[:], in_=xt_d[:, nind:nmid])
            if ntb > nmid:
                xbig = xp.tile([128, ntb - nmid, KCH, 128], f16,
                               tag="xbig", bufs=1)
                nc.sync.dma_start(out=xbig[:], in_=xt_d[:, nmid:])

            def get_xt(tb):
                if tb == 0:
                    return xt0
                if tb < nind:
                    return xtr[:, tb - 1]
                if tb < nmid:
                    return xmid[:, tb - nind]
                return xbig[:, tb - nmid]

            def drain(tb, ps):
                ot = op.tile([128, OUT_F], f16, name=f"ot{tb}", tag="ot")
                nc.scalar.copy(ot[:, 0:512], ps[:, 0:512])
                nc.vector.tensor_copy(ot[:, 512:1024], ps[:, 512:1024])
                nc.scalar.dma_start(
                    out=out_d[tb * 128:(tb + 1) * 128, :], in_=ot[:])

            # PE p-state ramp on scratch (no input-DMA dependency).
            for _ in range(NWARM):
                nc.tensor.matmul(psa[:], sc[:, 0:128], sc[:, 128:640],
                                 start=True, stop=True,
                                 skip_group_check=True)

            if ntb >= 3:
                # interleave the first two blocks so the W stream stays
                # ahead of chunk consumption even at full clock
                pxt = [get_xt(0), get_xt(1)]
                pps = [psp.tile([128, OUT_F], f32, name=f"psp{t}", tag="ps")
                       for t in range(2)]
                for k in range(KCH):
                    for t in range(2):
                        for nb in range(2):
                            nc.tensor.matmul(
                                pps[t][:, nb * 512:(nb + 1) * 512],
                                pxt[t][:, k, :],
                                wt[:, k, nb * 512:(nb + 1) * 512],
                                start=(k == 0),
                                stop=(k == KCH - 1),
                            )
                    if k < 6:
                        # scratch pad: slows chunk consumption to the W
                        # stream's sem cadence so the PE stays gap-free
                        nc.tensor.matmul(psa[:], sc[:, 0:128], sc[:, 128:640],
                                         start=True, stop=True,
                                         skip_group_check=True)
                drain(0, pps[0])
                drain(1, pps[1])
                start_tb = 2
            else:
                start_tb = 0

            for tb in range(start_tb, ntb):
                xt = get_xt(tb)
                last = tb == ntb - 1
                if not last:
                    ps = psp.tile([128, OUT_F], f32, name="ps", tag="ps")
                    for k in range(KCH):
                        for nb in range(2):
                            nc.tensor.matmul(
                                ps[:, nb * 512:(nb + 1) * 512],
                                xt[:, k, :],
                                wt[:, k, nb * 512:(nb + 1) * 512],
                                start=(k == 0),
                                stop=(k == KCH - 1),
                            )
                    drain(tb, ps)
                else:
                    # last block: separate PSUM tiles per 512-wide group so
                    # group-b matmuls don't serialize behind group-a copies;
                    # drain group a while b computes, then b in 256-col
                    # pieces on two engines/queues.
                    ot = op.tile([128, OUT_F], f16, name="otl", tag="ot")
                    r0, r1 = tb * 128, (tb + 1) * 128
                    for nb, pst in ((0, psa), (1, psb)):
                        for k in range(KCH):
                            nc.tensor.matmul(
                                pst[:],
                                xt[:, k, :],
                                wt[:, k, nb * 512:(nb + 1) * 512],
                                start=(k == 0),
                                stop=(k == KCH - 1),
                            )
                    # group a drains on the idle sync queue while group b
                    # computes, so the final (critical) group-b store has
                    # the warm scalar queue to itself
                    nc.scalar.copy(ot[:, 0:512], psa[:])
                    nc.sync.dma_start(out=out_d[r0:r1, 0:512],
                                      in_=ot[:, 0:512])
                    nc.scalar.copy(ot[:, 512:1024], psb[:])
                    nc.scalar.dma_start(out=out_d[r0:r1, 512:1024],
                                        in_=ot[:, 512:1024])
    nc.compile()
    return nc


def _pack_core(x_slice: np.ndarray, w_g: np.ndarray, ntb: int):
    n = x_slice.shape[0]
    xp = np.zeros((ntb * 128, IN_F), dtype=np.float16)
    xp[:n] = x_slice
    # [p, tb, k, tok]
    xt = np.ascontiguousarray(
        xp.reshape(ntb, 128, KCH, 128).transpose(3, 0, 2, 1)
    )
    wt = np.ascontiguousarray(
        w_g.astype(np.float16).reshape(KCH, 128, OUT_F).transpose(1, 0, 2)
    )
    return xt, wt


def kernel(hidden_states: np.ndarray, weight: np.ndarray, offsets: np.ndarray,
           _trace: bool = False):
    hs = np.ascontiguousarray(hidden_states, dtype=np.float32)
    w = np.ascontiguousarray(weight, dtype=np.float32)
    off = np.asarray(offsets).astype(np.int64)

    ends = np.clip(off, 0, TOKENS)
    starts = np.concatenate(([0], ends[:-1]))
    starts = np.minimum(starts, ends)
    ns = ends - starts

    ntb = max(1, int(-(-ns.max() // 128)))
    nc = build(ntb)

    in_maps = []
    for g in range(GROUPS):
        xt, wt = _pack_core(hs[starts[g]:ends[g]], w[g], ntb)
        in_maps.append({"xt": xt, "w": wt})

    res = run_bass_kernel_spmd(nc, in_maps, list(range(GROUPS)), trace=_trace)

    out = np.zeros((TOKENS, OUT_F), dtype=np.float32)
    for g in range(GROUPS):
        if ns[g] > 0:
            out[starts[g]:ends[g]] = res.results[g]["out"][:ns[g]].astype(
                np.float32)
    if _trace:
        return out, res
    return out



# revision 2
# speedup vs baseline: 1.0140x; 1.0140x over previous
"""Grouped (MoE-style) linear on 8 trn2 NeuronCores.

out[t] = hidden_states[t] @ weight[g(t)], where token t belongs to group g iff
offsets[g-1] <= t < offsets[g] (searchsorted right semantics; tokens at or past
offsets[-1] get zero output).

Strategy: expert-parallel. Core g owns weight[g] and the contiguous token run
of group g. Routing is done host-side (offsets are host data); each core runs
an identical Bass program: a [P_pad, 1024] x [1024, 1024] matmul tiled as
128-token blocks, contraction in 8 chunks of 128, PSUM-accumulated.

x and w are fed in fp16 (1 cycle/row PE rate like fp32r, half the HBM
traffic); PSUM accumulates fp32; output is written back as fp16 and upcast
host-side (measured end-to-end rel err ~4e-4).

Timeline model (from NTFF traces): fixed preamble + tile-context entry
barrier ends ~7.2us; vector memset ~7.45; DMA dispatch (DIRECT2D ~0.6-0.7us
each) starts ~7.2 on sync+scalar DGE queues, first data ~8.65, completion
sems lag data ~2us.  The fp16 PE stream floor is 256 MMs x 216ns = 55.3us,
so exec ~= chain_start + 55.3 + tail.  v2 changes vs the 76us baseline:
  - first-block inputs land earlier: xt0 split in two 128KB halves, w k0
    split in two 512-col halves (scalar queue), xt1 pulled out as its own
    DMA ahead of the bulk W pieces -> 2-block interleave can start ~12us.
  - pacing pads dropped except PADS right after the first k0 pair (bridges
    the xt1/W12 sem arrival); W chunk sems keep up with the 864ns/k
    consumption cadence of the 2-block phase.
  - last block: group-b PSUM drained in 384+128 pieces on two engines
    (scalar copy + vector copy) and two DGE queues (sync + scalar) so the
    exec-critical final store is 32KB instead of 128KB.

Host packs per-core inputs so DMA lands with wide contiguous runs per
SBUF partition:
  xt[p, tb, k, tok] = X_g[tb*128 + tok, k*128 + p]   (transposed token block)
  w[p, k, n]        = W_g[k*128 + p, n]
"""
import numpy as np

import concourse.bass as bass
import concourse.tile as tile
from concourse import bacc, mybir
from concourse.bass_utils import run_bass_kernel_spmd

GROUPS = 8
TOKENS = 16384
IN_F = 1024
OUT_F = 1024
KCH = IN_F // 128  # contraction chunks
NWARM = 10         # scratch ramp matmuls (cover ~7.7us -> data-ready ~12us)
PADS = 3           # scratch pads after first k0 pair (bridge xt1/W12 sems)


def build(ntb: int) -> bass.Bass:
    """One core's program: ntb 128-token blocks through a 1024x1024 expert."""
    f32 = mybir.dt.float32
    f16 = mybir.dt.float16
    nc = bacc.Bacc()
    xt_d = nc.dram_tensor("xt", [128, ntb, KCH, 128], f16, kind="ExternalInput")
    w_d = nc.dram_tensor("w", [128, KCH, OUT_F], f16, kind="ExternalInput")
    out_d = nc.dram_tensor("out", [ntb * 128, OUT_F], f16, kind="ExternalOutput")

    nind = min(4, ntb)   # token blocks in the early DMAs (xt0, xt1, xtr)
    nmid = min(8, ntb)   # token blocks covered by the first bulk DMA
    kh = KCH // 2
    with tile.TileContext(nc) as tc:
        with (
            tc.tile_pool(name="wp", bufs=1) as wp,
            tc.tile_pool(name="xp", bufs=nind) as xp,
            tc.tile_pool(name="op", bufs=4) as op,
            tc.tile_pool(name="sc", bufs=1) as scp,
            tc.tile_pool(name="ps", bufs=3, space="PSUM") as psp,
            tc.tile_pool(name="ph", bufs=1, space="PSUM") as php,
        ):
            wt = wp.tile([128, KCH, OUT_F], f16)
            sc = scp.tile([128, 640], f16)
            psa = php.tile([128, 512], f32, tag="psa")
            psb = php.tile([128, 512], f32, tag="psb")
            nc.vector.memset(sc[:], 0.0)

            # DMA queues are packet-rate-limited (~130-200 packets/us each,
            # one packet per contiguous per-partition run) and ~390 GB/s
            # aggregate.  Order matters: only the pieces the first ~7us of
            # the MM chain consumes go first, smallest-first so their
            # completion sems (data + ~2us) fire before consumption.
            # sync: xt0(k0-3), xt0(k4-7), xt1, W(k1-2), W(k3-4), W(k5-7),
            #       xt2-3, xt4-7, xt8+.   scalar: w k0 in two 512-col halves
            # (first real MM only needs xt0a + w0a), then output stores.
            xt0 = xp.tile([128, KCH, 128], f16, tag="xt", bufs=1)
            nc.sync.dma_start(out=xt0[:, 0:kh], in_=xt_d[:, 0, 0:kh])
            nc.scalar.dma_start(out=wt[:, 0, 0:512], in_=w_d[:, 0, 0:512])
            nc.scalar.dma_start(out=wt[:, 0, 512:OUT_F],
                                in_=w_d[:, 0, 512:OUT_F])
            nc.sync.dma_start(out=xt0[:, kh:KCH], in_=xt_d[:, 0, kh:KCH])
            if ntb > 1:
                xt1 = xp.tile([128, KCH, 128], f16, tag="xt1", bufs=1)
                nc.sync.dma_start(out=xt1[:], in_=xt_d[:, 1])
            nc.sync.dma_start(out=wt[:, 1:3, :], in_=w_d[:, 1:3, :])
            nc.sync.dma_start(out=wt[:, 3:5, :], in_=w_d[:, 3:5, :])
            nc.sync.dma_start(out=wt[:, 5:, :], in_=w_d[:, 5:, :])
            if nind > 2:
                xtr = xp.tile([128, nind - 2, KCH, 128], f16,
                              tag="xtr", bufs=1)
                nc.sync.dma_start(out=xtr[:], in_=xt_d[:, 2:nind])
            if nmid > nind:
                xmid = xp.tile([128, nmid - nind, KCH, 128], f16,
                               tag="xmid", bufs=1)
                nc.sync.dma_start(out=xmid[:], in_=xt_d[:, nind:nmid])
            if ntb > nmid:
                xbig = xp.tile([128, ntb - nmid, KCH, 128], f16,
                               tag="xbig", bufs=1)
                nc.sync.dma_start(out=xbig[:], in_=xt_d[:, nmid:])

            def get_xt(tb):
                if tb == 0:
                    return xt0
                if tb == 1:
                    return xt1
                if tb < nind:
                    return xtr[:, tb - 2]
                if tb < nmid:
                    return xmid[:, tb - nind]
                return xbig[:, tb - nmid]

            def drain(tb, ps):
                ot = op.tile([128, OUT_F], f16, name=f"ot{tb}", tag="ot")
                nc.scalar.copy(ot[:, 0:512], ps[:, 0:512])
                nc.vector.tensor_copy(ot[:, 512:1024], ps[:, 512:1024])
                nc.scalar.dma_start(
                    out=out_d[tb * 128:(tb + 1) * 128, :], in_=ot[:])

            def pad(n):
                for _ in range(n):
                    nc.tensor.matmul(psa[:], sc[:, 0:128], sc[:, 128:640],
                                     start=True, stop=True,
                                     skip_group_check=True)

            # PE HAM ramp on scratch (no input-DMA dependency): PE busy from
            # ~7.7us so the clock is warm (2.4GHz) before real MMs start.
            pad(NWARM)

            if ntb >= 3:
                # interleave the first two blocks: 4 MMs per k chunk = 864ns
                # cadence, matched to the W chunk sem arrival rate
                pxt = [get_xt(0), get_xt(1)]
                pps = [psp.tile([128, OUT_F], f32, name=f"psp{t}", tag="ps")
                       for t in range(2)]
                for k in range(KCH):
                    for t in range(2):
                        for nb in range(2):
                            nc.tensor.matmul(
                                pps[t][:, nb * 512:(nb + 1) * 512],
                                pxt[t][:, k, :],
                                wt[:, k, nb * 512:(nb + 1) * 512],
                                start=(k == 0),
                                stop=(k == KCH - 1),
                            )
                        if k == 0 and t == 0:
                            pad(PADS)  # xt1/W12 sems land ~0.5-1us after xt0
                drain(0, pps[0])
                drain(1, pps[1])
                start_tb = 2
            else:
                start_tb = 0

            for tb in range(start_tb, ntb):
                xt = get_xt(tb)
                last = tb == ntb - 1
                if not last:
                    ps = psp.tile([128, OUT_F], f32, name="ps", tag="ps")
                    for k in range(KCH):
                        for nb in range(2):
                            nc.tensor.matmul(
                                ps[:, nb * 512:(nb + 1) * 512],
                                xt[:, k, :],
                                wt[:, k, nb * 512:(nb + 1) * 512],
                                start=(k == 0),
                                stop=(k == KCH - 1),
                            )
                    drain(tb, ps)
                else:
                    # last block: separate PSUM tiles per 512-wide group so
                    # group-b matmuls don't serialize behind group-a copies.
                    # Group a drains on the sync queue while b computes; b
                    # drains in 384+128 pieces on two engines + two queues
                    # so the exec-critical final store is only 32KB.
                    ot = op.tile([128, OUT_F], f16, name="otl", tag="ot")
                    r0, r1 = tb * 128, (tb + 1) * 128
                    for nb, pst in ((0, psa), (1, psb)):
                        for k in range(KCH):
                            nc.tensor.matmul(
                                pst[:],
                                xt[:, k, :],
                                wt[:, k, nb * 512:(nb + 1) * 512],
                                start=(k == 0),
                                stop=(k == KCH - 1),
                            )
                    nc.scalar.copy(ot[:, 0:512], psa[:])
                    nc.sync.dma_start(out=out_d[r0:r1, 0:512],
                                      in_=ot[:, 0:512])
                    nc.scalar.copy(ot[:, 512:896], psb[:, 0:384])
                    nc.vector.tensor_copy(ot[:, 896:1024], psb[:, 384:512])
                    nc.sync.dma_start(out=out_d[r0:r1, 512:896],
                                      in_=ot[:, 512:896])
                    nc.scalar.dma_start(out=out_d[r0:r1, 896:1024],
                                        in_=ot[:, 896:1024])
    nc.compile()
    return nc


def _pack_core(x_slice: np.ndarray, w_g: np.ndarray, ntb: int):
    n = x_slice.shape[0]
    xp = np.zeros((ntb * 128, IN_F), dtype=np.float16)
    xp[:n] = x_slice
    # [p, tb, k, tok]
    xt = np.ascontiguousarray(
        xp.reshape(ntb, 128, KCH, 128).transpose(3, 0, 2, 1)
    )
    wt = np.ascontiguousarray(
        w_g.astype(np.float16).reshape(KCH, 128, OUT_F).transpose(1, 0, 2)
    )
    return xt, wt


def kernel(hidden_states: np.ndarray, weight: np.ndarray, offsets: np.ndarray,
           _trace: bool = False):
    hs = np.ascontiguousarray(hidden_states, dtype=np.float32)
    w = np.ascontiguousarray(weight, dtype=np.float32)
    off = np.asarray(offsets).astype(np.int64)

    ends = np.clip(off, 0, TOKENS)
    starts = np.concatenate(([0], ends[:-1]))
    starts = np.minimum(starts, ends)
    ns = ends - starts

    ntb = max(1, int(-(-ns.max() // 128)))
    nc = build(ntb)

    in_maps = []
    for g in range(GROUPS):
        xt, wt = _pack_core(hs[starts[g]:ends[g]], w[g], ntb)
        in_maps.append({"xt": xt, "w": wt})

    res = run_bass_kernel_spmd(nc, in_maps, list(range(GROUPS)), trace=_trace)

    out = np.zeros((TOKENS, OUT_F), dtype=np.float32)
    for g in range(GROUPS):
        if ns[g] > 0:
            out[starts[g]:ends[g]] = res.results[g]["out"][:ns[g]].astype(
                np.float32)
    if _trace:
        return out, res
    return out
